# revision 47
# baseline (speedup 1.0000x reference)
"""Multi-head attention (B=16, GS=1024, E=768, H=12, D=64) on 8 trn2 NeuronCores.

Sharding: data-parallel over batch - 2 batches per core, no collectives.

Per-core design (per batch of S=1024 tokens):
  1. qkT = (x @ w_qk)^T -> [2E, S] (head-dim on partitions); q gets its bias
     (the k bias is softmax-invariant and dropped), v = x @ w_v in natural
     layout plus a ones column per head (v bias is folded into the proj bias
     on the host: b_eff = b_proj + b_v @ w_proj).
  2. heads in pairs (two 64-dim heads share the 128 PE rows via tile_position
     row groups): scoresT[ki,qi] matmuls, exp on ACT (scale=1/8 fused, no max
     subtraction - scores ~ N(0,1)).
  3. FLIPPED PV: out[q, d] = ep_chunk^T @ v_chunk - the exp'd scores are the
     stationary operand [128 keys, 128 q] and v the moving one [128, 65]
     (65-cycle streams; measured ~44ns/instr incl. hidden weight loads vs
     216ns for the v-stationary form). Denominator = ones column of v, lands
     per-PARTITION, so normalization is a plain DVE reciprocal +
     tensor_scalar multiply (no PE broadcast matmuls).
  4. The normalized [q, d] tiles are transposed back to attnT [d-pair, q]
     layout by the DMA engine's XBAR (dma_start_transpose, 16x128 tiles) -
     zero PE cost.
  5. proj: y = attnT^T @ w_proj + b_eff per 128-column chunk.

PSUM: scores [P,2,512]x2 (4 banks) + pv [P,2,2,65]x2 padded to a bank each
(two 2-series groups per bank share one start=True bank-zero) + prep/proj
[P,512]x2 (2 banks) = 8 banks exactly.

Weights stream per 128-column chunk in the exact order the first matmuls
consume them. Batch b+1's prep (v first, then qk pairs 0-3) fills batch b's
head phase; qk pairs 4-5 lag into batch 1's own early head groups; batch 0's
proj and batch 1's qi0-proj fill batch 1's later head groups (heads run
qi-major so qi0's attnT completes halfway through).
"""

import numpy as np
from contextlib import ExitStack

import concourse.bass as bass
import concourse.mybir as mybir
import concourse.tile as tile
from concourse import bacc

F32 = mybir.dt.float32
BF16 = mybir.dt.bfloat16
AF = mybir.ActivationFunctionType
P = 128


def build_nc(BPC=2, S=1024, E=768, H=12, D=64, act_dtype=BF16):
    SCALE = D ** -0.5
    EC = E // P              # emb chunks (6)
    SC = S // P              # seq chunks per batch (8)
    QT = 512                 # qi tile size
    NQT = S // QT            # qi tiles per batch (2)
    HPC = P // D             # heads per 128-chunk (pair size, 2)
    NPAIR = H // HPC         # 6
    T = BPC * S
    DV = D + 1               # v columns incl. ones
    NQC = QT // P            # 128-token chunks per qi tile (4)

    nc = bacc.Bacc("TRN2", target_bir_lowering=False, debug=False)

    NG = T // QT  # 512-token groups per core
    # x pre-arranged on host: [p, g, ec, t'] = x^T[ec*128+p, g*512+t']
    x_d = nc.dram_tensor("x_pre", [P, NG, EC, QT], act_dtype,
                         kind="ExternalInput")
    # q,k weight columns pre-arranged on host: [p, c, ec] = w_qkv[ec*128+p, c]
    wqk_d = nc.dram_tensor("w_qk_pre", [P, 2 * E, EC], act_dtype,
                           kind="ExternalInput")
    wv_d = nc.dram_tensor("w_v", [E, E], act_dtype, kind="ExternalInput")
    bq_d = nc.dram_tensor("bq_pre", [P, EC], F32, kind="ExternalInput")
    wproj_d = nc.dram_tensor("w_proj", [E, E], act_dtype, kind="ExternalInput")
    beff_d = nc.dram_tensor("beff_pre", [P, EC], F32, kind="ExternalInput")
    y_d = nc.dram_tensor("y_local", [E, T], act_dtype, kind="ExternalOutput")

    # column-chunk order for the qk part of wqkv: q chunk then its k partner
    M_ORDER = []
    for i in range(EC):
        M_ORDER += [i, EC + i]

    with tile.TileContext(nc) as tc, ExitStack() as ctx:
        const = ctx.enter_context(tc.tile_pool(name="const", bufs=1))
        xtp = ctx.enter_context(tc.tile_pool(name="xtp", bufs=1))
        qkp = ctx.enter_context(tc.tile_pool(name="qkp", bufs=2))
        vp = ctx.enter_context(tc.tile_pool(name="vp", bufs=2))
        atp = ctx.enter_context(tc.tile_pool(name="atp", bufs=2))
        expp = ctx.enter_context(tc.tile_pool(name="expp", bufs=2))
        outp = ctx.enter_context(tc.tile_pool(name="outp", bufs=2))
        stgp = ctx.enter_context(tc.tile_pool(name="stgp", bufs=2))
        ps_sc = ctx.enter_context(tc.tile_pool(name="ps_sc", bufs=2, space="PSUM"))
        ps_pv = ctx.enter_context(tc.tile_pool(name="ps_pv", bufs=2, space="PSUM"))
        ps_pr = ctx.enter_context(tc.tile_pool(name="ps_pr", bufs=2, space="PSUM"))

        # ---------------- constants / weights ----------------
        wqk_sb = const.tile([P, 2 * E, EC], act_dtype, name="wqk_sb")
        wv_sb = const.tile([P, EC, E], act_dtype, name="wv_sb")
        wproj_sb = const.tile([P, EC, E], act_dtype, name="wproj_sb")
        bq_sb = const.tile([P, EC], F32)
        beff_sb = const.tile([P, EC], F32)
        wrm = const.tile([P, 2 * P], act_dtype, name="wrm")
        warm = const.tile([P, 1], F32)

        def u_setup():
            nc.scalar.dma_start(bq_sb, bq_d.ap())
            nc.vector.memset(warm, 0.0)
            nc.scalar.activation(warm, warm, AF.Exp, scale=1.0)
            nc.vector.memset(wrm, 0.0)

        def u_warm_pe():
            # dummy matmuls during the DMA-startup dead time: trigger the HAM
            # duty-cycle boost before the real matmul stream begins
            dum = ps_sc.tile([P, HPC, 512], F32, tag="sc", name="dum")
            for _ in range(48):
                nc.tensor.matmul(dum[:, 0, 0:2 * P], wrm[:, 0:P], wrm,
                                 start=True, stop=True)

        def u_setup2():
            # effective proj bias (needed late, at first proj)
            nc.scalar.dma_start(beff_sb, beff_d.ap())

        def wq_load(m):
            nc.sync.dma_start(wqk_sb[:, m * P:(m + 1) * P, :],
                              wqk_d[:, m * P:(m + 1) * P, :])

        def wv_load(ec):
            nc.gpsimd.dma_start(wv_sb[:, ec, :], wv_d[ec * P:(ec + 1) * P, :])

        def wp_load(ec):
            nc.gpsimd.dma_start(wproj_sb[:, ec, :],
                                wproj_d[ec * P:(ec + 1) * P, :])

        states = {}

        # ---------------- unit builders ----------------
        def u_alloc(b):
            st = states.setdefault(b, {})

            def u():
                st["xT"] = [xtp.tile([P, EC, QT], act_dtype, name=f"xT{b}_{qi}",
                                     tag=f"xT{qi}") for qi in range(NQT)]
                st["qkT"] = qkp.tile([P, 2 * EC, S], act_dtype, name=f"qkT{b}",
                                     tag="qkT")
                st["v"] = vp.tile([P, SC, H, DV], act_dtype, name=f"v{b}", tag="v")
                st["attnT"] = atp.tile([P, EC, S], act_dtype, name=f"attnT{b}",
                                       tag="attnT")
                nc.vector.memset(st["v"][:, :, :, D:DV], 1.0)
            return u

        def u_xdma(b, qi, split=False):
            # split: halves on separate queues so the first qk matmul can
            # start as soon as ec 0..2 land
            def u():
                st = states[b]
                if split:
                    nc.sync.dma_start(st["xT"][qi][:, 0:3, :],
                                      x_d[:, b * NQT + qi, 0:3, :])
                    nc.scalar.dma_start(st["xT"][qi][:, 3:EC, :],
                                        x_d[:, b * NQT + qi, 3:EC, :])
                else:
                    nc.sync.dma_start(st["xT"][qi][:, :, :],
                                      x_d[:, b * NQT + qi, :, :])
            return u

        def u_qk(b, m, qi):
            def u():
                st = states[b]
                pt = ps_pr.tile([P, 512], F32, tag="pr", name=f"qk{b}_{m}_{qi}")
                for ec in range(EC):
                    nc.tensor.matmul(
                        pt[:, 0:QT],
                        wqk_sb[:, m * P:(m + 1) * P, ec],
                        st["xT"][qi][:, ec, :],
                        start=(ec == 0), stop=(ec == EC - 1),
                    )
                dst = st["qkT"][:, m, qi * QT:(qi + 1) * QT]
                if m < EC:   # q chunk: add bias
                    nc.vector.tensor_scalar_add(dst, pt[:, 0:QT], bq_sb[:, m:m + 1])
                else:        # k chunk: bias dropped (softmax-invariant)
                    nc.vector.tensor_copy(dst, pt[:, 0:QT])
            return u

        V_NTS = [(0, 512), (512, 256)]

        def u_v(b, si, k):
            def u():
                st = states[b]
                nt, n_sl = V_NTS[k]
                pt = ps_pr.tile([P, 512], F32, tag="pr", name=f"v{b}_{si}_{k}")
                qi, so = divmod(si * P, QT)
                for ec in range(EC):
                    nc.tensor.matmul(
                        pt[:, 0:n_sl],
                        st["xT"][qi][:, ec, so:so + P],
                        wv_sb[:, ec, nt:nt + n_sl],
                        start=(ec == 0), stop=(ec == EC - 1),
                    )
                nc.vector.tensor_copy(
                    st["v"][:, si, nt // D: (nt + n_sl) // D, 0:D],
                    pt[:, 0:n_sl].rearrange("p (h d) -> p h d", d=D))
            return u

        def u_sc_exp(b, pr, qi, kc, ep):
            def u():
                st = states[b]
                qkT = st["qkT"]
                ps = ps_sc.tile([P, HPC, 512], F32, tag="sc")
                for j in range(HPC):
                    po = D * j
                    nc.tensor.matmul(
                        ps[:, j, 0:QT],
                        qkT[po:po + D, EC + pr, kc * P:(kc + 1) * P],
                        qkT[po:po + D, pr, qi * QT:(qi + 1) * QT],
                        start=True, stop=True,
                        tile_position=(po, 0),
                    )
                nc.scalar.activation(ep[:, kc, :, :], ps[:, :, 0:QT],
                                     AF.Exp, scale=SCALE)
            return u

        def u_pv_kc(b, pr, qi, kc, banks, ep):
            # flipped PV: stationary = ep [128 keys, 128 q-cols], moving =
            # v [128 keys, 65] (64 d + ones col -> denominator at col 64).
            # Each psum bank holds two q-chunks x two heads; only the very
            # first matmul into a bank uses start=True (bank zero), the other
            # three series accumulate onto the pending-zeroed bank.
            def u():
                st = states[b]
                for qc in range(NQC):
                    bank = banks[qc // 2]
                    qcin = qc % 2
                    for j in range(HPC):
                        h = pr * HPC + j
                        nc.tensor.matmul(
                            bank[:, qcin, j, :],
                            ep[:, kc, j, qc * P:(qc + 1) * P],
                            st["v"][:, kc, h, :],
                            start=(kc == 0 and qcin == 0 and j == 0),
                            stop=(kc == SC - 1),
                            skip_group_check=True,
                        )
            return u

        def u_drain(b, pr, qi, banks, stg, rcpt):
            # reciprocal of the two denominator columns, then per-partition
            # scalar multiply draining psum -> bf16 staging [q, (qc, j, d)]
            def u():
                for qc in range(NQC):
                    bank = banks[qc // 2]
                    qcin = qc % 2
                    nc.vector.reciprocal_approx_fast(
                        rcpt[:, qc, :], bank[:, qcin, :, D])
                    for j in range(HPC):
                        nc.vector.tensor_scalar_mul(
                            stg[:, qc, j, :],
                            bank[:, qcin, j, 0:D],
                            rcpt[:, qc, j:j + 1])
            return u

        def u_transpose(b, pr, qi, stg):
            # XBAR block-transpose: staging [128 q', 4*(2*64)] ->
            # attnT[:, pr, qi*512 + qc*128 + q'] with partitions = (j, d)
            def u():
                st = states[b]
                dst = st["attnT"][:, pr, qi * QT:(qi + 1) * QT]
                nc.sync.dma_start_transpose(
                    dst.rearrange("p (g t) -> p g t", t=P),
                    stg.rearrange("p a b c -> p (a b c)"))
            return u

        def u_proj(b, mo, qi, alt=False, dma_eng=None, on_act=False):
            # transposed proj: yT[e_out, tok] = w_projT-chunk @ attnT-columns;
            # bias is per-partition, so the PSUM drain is a single fused
            # bias+copy
            def u():
                st = states[b]
                if alt:  # tail: borrow the idle scores psum pool
                    pt = ps_sc.tile([P, HPC, 512], F32, tag="sc",
                                    name=f"pj{b}_{mo}_{qi}")[:, 0, :]
                else:
                    pt = ps_pr.tile([P, 512], F32, tag="pr",
                                    name=f"pj{b}_{mo}_{qi}")
                for ec in range(EC):
                    nc.tensor.matmul(
                        pt[:, 0:QT],
                        wproj_sb[:, ec, mo * P:(mo + 1) * P],
                        st["attnT"][:, ec, qi * QT:(qi + 1) * QT],
                        start=(ec == 0), stop=(ec == EC - 1),
                    )
                yt = outp.tile([P, 512], act_dtype, tag="y",
                               name=f"y{b}_{mo}_{qi}")
                if on_act:
                    nc.scalar.activation(yt, pt[:, 0:QT], AF.Identity,
                                         bias=beff_sb[:, mo:mo + 1])
                else:
                    nc.vector.tensor_scalar_add(yt, pt[:, 0:QT],
                                                beff_sb[:, mo:mo + 1])
                eng = dma_eng if dma_eng is not None else nc.sync
                eng.dma_start(
                    y_d[mo * P:(mo + 1) * P,
                        b * S + qi * QT: b * S + (qi + 1) * QT],
                    yt)
            return u

        proj_parts = {}

        def u_proj_part(b, mo, qi, alt=False):
            # first 4 contraction terms of a proj chain (pairs 0..3 attnT is
            # ready early): runs in the last head group's idle, holding its
            # psum bank until the finisher
            def u():
                st = states[b]
                if alt:
                    pt = ps_sc.tile([P, HPC, 512], F32, tag="sc",
                                    name=f"pp{b}_{mo}_{qi}")[:, 0, :]
                else:
                    pt = ps_pr.tile([P, 512], F32, tag="pr",
                                    name=f"pp{b}_{mo}_{qi}")
                for ec in range(4):
                    nc.tensor.matmul(
                        pt[:, 0:QT],
                        wproj_sb[:, ec, mo * P:(mo + 1) * P],
                        st["attnT"][:, ec, qi * QT:(qi + 1) * QT],
                        start=(ec == 0), stop=False,
                    )
                proj_parts[(b, mo, qi)] = pt
            return u

        def u_proj_fin(b, mo, qi, dma_eng, on_act):
            def u():
                st = states[b]
                pt = proj_parts[(b, mo, qi)]
                for ec in (4, 5):
                    nc.tensor.matmul(
                        pt[:, 0:QT],
                        wproj_sb[:, ec, mo * P:(mo + 1) * P],
                        st["attnT"][:, ec, qi * QT:(qi + 1) * QT],
                        start=False, stop=(ec == EC - 1),
                        skip_group_check=True,
                    )
                yt = outp.tile([P, 512], act_dtype, tag="y",
                               name=f"yf{b}_{mo}_{qi}")
                if on_act:
                    nc.scalar.activation(yt, pt[:, 0:QT], AF.Identity,
                                         bias=beff_sb[:, mo:mo + 1])
                else:
                    nc.vector.tensor_scalar_add(yt, pt[:, 0:QT],
                                                beff_sb[:, mo:mo + 1])
                dma_eng.dma_start(
                    y_d[mo * P:(mo + 1) * P,
                        b * S + qi * QT: b * S + (qi + 1) * QT],
                    yt)
            return u

        def qk_pair_units(b, pr):
            return [u_qk(b, m, qi) for m in (pr, EC + pr) for qi in range(NQT)]

        def v_units(b):
            return [u_v(b, si, k) for si in range(SC) for k in range(2)]

        def head_group(b, pr, qi, fillers, budget, last=False, post_pv=()):
            """Emit one (pair, qi) head group, popping `budget` fillers."""
            pops = [False] * 5
            for i in range(min(budget, 5)):
                pops[i % 5] = True
            if last:
                # score-phase fillers at kc1/kc3 plus the post-transpose
                # window (PE otherwise idles there waiting on final attnT)
                pops = [True, True, False, False, True]
            ep = expp.tile([P, SC, HPC, QT], act_dtype, tag="exp",
                           name=f"ep{b}_{pr}_{qi}")
            # scores first (PE ~432ns/kc vs ACT exp ~1066ns/kc): fillers
            # plug the PE idle inside the score stream
            for kc in range(SC):
                u_sc_exp(b, pr, qi, kc, ep)()
                if kc in (1, 3, 5) and pops[kc // 2] and fillers:
                    fillers.pop(0)()
            banks = [ps_pv.tile([P, 2, HPC, DV], F32, tag="pv",
                                padded_shape=[P, 2, HPC, 128],
                                name=f"pv{b}_{pr}_{qi}_{hb}")
                     for hb in range(2)]
            stg = stgp.tile([P, NQC, HPC, D], act_dtype, tag="stg",
                            name=f"stg{b}_{pr}_{qi}")
            rcpt = stgp.tile([P, NQC, HPC], F32, tag="rcp",
                             name=f"rcp{b}_{pr}_{qi}")
            for kc in range(SC):
                u_pv_kc(b, pr, qi, kc, banks, ep)()
                if kc == 3 and pops[3] and fillers:
                    fillers.pop(0)()
                if kc == SC - 2 and post_pv:
                    # fills the PE idle while pv waits on the final exp
                    post_pv[0]()
            for u in post_pv[1:]:
                u()
            u_drain(b, pr, qi, banks, stg, rcpt)()
            u_transpose(b, pr, qi, stg)()
            if last:
                while fillers:
                    fillers.pop(0)()
            elif pops[4] and fillers:
                fillers.pop(0)()

        # ---------------- emission schedule ----------------
        u_alloc(0)()
        u_xdma(0, 0, split=True)()
        wq_load(M_ORDER[0])
        wq_load(M_ORDER[1])
        # x qi1 on the (software-DGE) gpsimd queue: parallel to the sync
        # queue's weight-chunk stream
        st0 = states[0]
        nc.gpsimd.dma_start(st0["xT"][1][:, :, :], x_d[:, 1, :, :])
        u_setup()
        u_warm_pe()
        for m in M_ORDER[2:]:
            wq_load(m)
        for ec in range(EC):
            wv_load(ec)
        u_setup2()
        for ec in range(EC):
            wp_load(ec)
        for m in M_ORDER:
            for qi in range(NQT):
                u_qk(0, m, qi)()
        for u in v_units(0):
            u()

        # batch 0 heads (qi-major), filled with batch 1 prep: v first (needed
        # at batch 1's first pv), then qk pairs 0..3
        u_alloc(1)()
        fillers = [u_xdma(1, 0), u_xdma(1, 1)] + v_units(1)
        for pr in range(4):
            fillers += qk_pair_units(1, pr)
        ngroups = NPAIR * NQT
        for g, (qi, pr) in enumerate((qi, pr) for qi in range(NQT)
                                     for pr in range(NPAIR)):
            budget = -(-len(fillers) // (ngroups - g))
            head_group(0, pr, qi, fillers, budget)
        for u in fillers:
            u()

        # batch 1 heads (qi-major): early groups fill with batch 1's
        # remaining qk pairs (4, 5), later groups with batch 0 proj and
        # batch 1 qi0 proj
        fillers = qk_pair_units(1, 4) + qk_pair_units(1, 5)
        fillers += [u_proj(0, mo, qi) for qi in range(NQT)
                    for mo in range(EC)]
        for g, (qi, pr) in enumerate((qi, pr) for qi in range(NQT)
                                     for pr in range(NPAIR)):
            if qi == 1 and pr == 0:
                # batch 1 qi0 attnT is complete once its transposes land
                fillers += [u_proj(1, mo, 0) for mo in range(EC)]
            last = qi == 1 and pr == NPAIR - 1
            if last:
                # the final group's score-phase fillers are ec0..3 proj
                # partials (pairs 0..3 attnT landed two groups ago); two more
                # partials borrow the freed score psum right after the pv
                # block. Their held banks turn the post-transpose tail into
                # four short 2-term finishers + two full chains.
                assert not fillers, "heads1 fillers must drain before last"
                fillers = [u_proj_part(1, 0, 1), u_proj_part(1, 1, 1)]
                head_group(1, pr, qi, fillers, 2, last=True,
                           post_pv=(u_proj_part(1, 2, 1, alt=True),
                                    u_proj_part(1, 3, 1, alt=True)))
            else:
                # lean early (2/group), drain everything by group 10
                budget = min(2 if g < 6 else 3, len(fillers))
                head_group(1, pr, qi, fillers, budget)

        # batch 1 qi1 proj tail: 4 finishers on held psum + 2 full chains,
        # alternating bias engine (ACT/DVE) and DMA sequencer (sync/scalar)
        for n, mo in enumerate(range(4)):
            u_proj_fin(1, mo, 1,
                       dma_eng=(nc.scalar if n % 2 == 1 else nc.sync),
                       on_act=(n % 2 == 0))()
        u_proj(1, 4, 1, alt=False, dma_eng=nc.sync, on_act=True)()
        u_proj(1, 5, 1, alt=True, dma_eng=nc.scalar, on_act=False)()

    nc.compile()
    return nc


_NC_CACHE = {}


def _get_nc():
    if "nc" not in _NC_CACHE:
        _NC_CACHE["nc"] = build_nc()
    return _NC_CACHE["nc"]


B, GS, E_FULL = 16, 1024, 768
N_CORES = 8
BPC_FULL = B // N_CORES


def make_in_maps(x, w_qkv, b_qkv, w_proj, b_proj):
    import ml_dtypes
    bf = ml_dtypes.bfloat16
    x = np.asarray(x, dtype=np.float32).astype(bf)  # [B, GS, E]
    w_qkv_f = np.asarray(w_qkv, dtype=np.float32)
    b_qkv_f = np.asarray(b_qkv, dtype=np.float32)
    w_proj_f = np.asarray(w_proj, dtype=np.float32)
    b_proj_f = np.asarray(b_proj, dtype=np.float32)
    # fold the v bias through the projection: out = (attn + b_v) @ w_proj + b
    b_eff = (b_proj_f.astype(np.float64)
             + b_qkv_f[2 * E_FULL:].astype(np.float64)
             @ w_proj_f.astype(np.float64)).astype(np.float32)
    w_qkv_b = w_qkv_f.astype(bf)
    # q,k columns pre-arranged: [p, c, ec] = w_qkv[ec*128+p, c]
    w_qk_pre = np.ascontiguousarray(
        w_qkv_b[:, :2 * E_FULL].reshape(6, 128, 2 * E_FULL).transpose(1, 2, 0))
    w_v = np.ascontiguousarray(w_qkv_b[:, 2 * E_FULL:])
    w_proj_b = np.ascontiguousarray(w_proj_f.astype(bf))
    # q bias as [p, ec] = b_qkv[ec*128 + p]
    bq_pre = np.ascontiguousarray(b_qkv_f[:E_FULL].reshape(6, 128).T)
    beff_pre = np.ascontiguousarray(b_eff.reshape(6, 128).T)
    in_maps = []
    T = BPC_FULL * GS
    for i in range(N_CORES):
        xt = x[i * BPC_FULL:(i + 1) * BPC_FULL].reshape(T, E_FULL).T  # [E, T]
        # [p, g, ec, t'] = xt[ec*128+p, g*512+t']
        x_pre = np.ascontiguousarray(
            xt.reshape(6, 128, T // 512, 512).transpose(1, 2, 0, 3))
        in_maps.append({
            "x_pre": x_pre,
            "w_qk_pre": w_qk_pre, "w_v": w_v, "bq_pre": bq_pre,
            "w_proj": w_proj_b, "beff_pre": beff_pre,
        })
    return in_maps


def gather_out(results):
    return np.concatenate(
        [r["y_local"].T.reshape(BPC_FULL, GS, E_FULL) for r in results],
        axis=0).astype(np.float32)


def kernel(x, w_qkv, b_qkv, w_proj, b_proj):
    from concourse.bass_utils import run_bass_kernel_spmd

    nc = _get_nc()
    in_maps = make_in_maps(x, w_qkv, b_qkv, w_proj, b_proj)
    res = run_bass_kernel_spmd(nc, in_maps, core_ids=list(range(N_CORES)))
    return gather_out(res.results)
